# revision 49
# baseline (speedup 1.0000x reference)
"""Trainium2 Bass kernel for nn_DepthSeparableConv2d_conv2_5.

Computation (per sample):
  y = relu(BN1(depthwise3x3(x) + dw_b));  y = prune(y, 4.0)   [per-(b,c) absmax]
  z = relu(BN2(pw_w @ y + pw_b));         z = prune(z, 0.001) [per-(b,o) absmax]

Mapping (8 NeuronCores, data-parallel over batch, 8 samples/core):
  - depthwise conv = 9 accumulating fp32r matmuls with diagonal [128,128]
    weight matrices (one per tap); the spatial shift is an access-pattern
    offset on the rhs (x rows are host-padded 56->57 with a shared zero
    column so every tap is a full even-width region, as fp32r requires).
  - DW epilogue (BN1 scale+bias+ReLU) on ScalarE, one PSUM bank per op.
  - exact per-plane maxes via DVE tensor_scalar with accum_out (op1=max).
  - DW prune mask is folded into the pointwise lhsT (zero pruned rows).
  - pointwise 1x1 conv = fp32r GEMM, BN2 scale folded into the pw weights,
    bias+ReLU on ScalarE, prune mask applied on DVE before the store.
"""

import numpy as np

import concourse.bass as bass
import concourse.mybir as mybir
import concourse.tile as tile
from concourse import bacc
from concourse.bass_utils import run_bass_kernel_spmd
from concourse.masks import make_identity

f32 = mybir.dt.float32
f32r = mybir.dt.float32r
Alu = mybir.AluOpType
Act = mybir.ActivationFunctionType
AxL = mybir.AxisListType

N_CORES = 8
B = 64
BPC = B // N_CORES  # samples per core
CIN, COUT = 128, 256
H = W = 56
HW = H * W   # 3136
WP = W + 1   # host-padded row width: one zero col; dw=+1 wraps onto the
             # NEXT row's pad col (also zero), so one col serves both sides
HWP = H * WP + 2  # +2 trailing zeros so the (h=55, dw=+1) wrap view fits  # 3194
NT = 7       # pixel tiles per plane, 8 rows (448 px) each
TR = 8       # rows per pixel tile
EPS = 1e-5
DW_T, PW_T = 4.0, 0.001

# tap (0,0) first: it covers the full region, so it carries start=True
TAPS = [(0, 0)] + [
    (dh, dw) for dh in (-1, 0, 1) for dw in (-1, 0, 1) if (dh, dw) != (0, 0)
]


def _affine(nc, pool, name, var, gamma, beta, mean, bvec):
    """s = gamma/sqrt(var+eps); t = (bvec - mean)*s + beta. All [128,1]."""
    tmp = pool.tile([128, 1], f32, tag=f"{name}_tmp")
    nc.vector.tensor_scalar(tmp[:], var, EPS, None, Alu.add)
    sq = pool.tile([128, 1], f32, tag=f"{name}_sq")
    nc.scalar.sqrt(sq[:], tmp[:])
    rc = pool.tile([128, 1], f32, tag=f"{name}_rc")
    nc.vector.reciprocal(rc[:], sq[:])
    s = pool.tile([128, 1], f32, tag=f"{name}_s")
    nc.vector.tensor_mul(s[:], gamma, rc[:])
    u = pool.tile([128, 1], f32, tag=f"{name}_u")
    nc.vector.tensor_sub(u[:], bvec, mean)
    nc.vector.tensor_mul(u[:], u[:], s[:])
    t = pool.tile([128, 1], f32, tag=f"{name}_t")
    nc.vector.tensor_add(t[:], u[:], beta)
    return s, t


def build():
    nc = bacc.Bacc(trn_type="TRN2", target_bir_lowering=False, debug=False)

    # x is consumed only by fp32r matmuls; declaring it float32r end-to-end
    # satisfies the verifier's "rounded to FP32r" producer rule with a plain
    # HWDGE same-dtype DMA (bits are ordinary fp32; the PE rounds on read).
    x_d = nc.dram_tensor("x", [BPC, CIN, HWP], f32r, kind="ExternalInput").ap()
    # every parameter packed into one [128, 280] tensor (single DMA):
    # cols 0:14   dw_w(9) | dw_b | bn1 gamma/beta/mean/var      (per channel)
    # cols 14:19  pw_b | bn2 gamma/beta/mean/var  for outputs 0:128
    # cols 19:24  same for outputs 128:256
    # cols 24:152 pw_w rows 0:128 | cols 152:280 pw_w rows 128:256
    pall_d = nc.dram_tensor("pall", [CIN, 280], f32, kind="ExternalInput").ap()
    z_d = nc.dram_tensor("z", [BPC, COUT, HW], f32, kind="ExternalOutput").ap()

    with tile.TileContext(nc) as tc:
        with (
            tc.tile_pool(name="const", bufs=1) as const,
            tc.tile_pool(name="stats", bufs=6) as stats,
            tc.tile_pool(name="xp", bufs=4) as xpool,
            tc.tile_pool(name="yp", bufs=4) as ypool,
            tc.tile_pool(name="zp", bufs=4) as zpool,
            tc.tile_pool(name="lmp", bufs=2) as lmpool,
        ):
            # ---------------- setup: params; dw_w columns go in a tiny
            # first DMA so the tap diagonals (first matmul's dependency)
            # don't wait for the full 280-column transfer ---------
            # sample 0 head start: rows for its first three DW groups land
            # in dedicated small tiles so group t doesn't wait on the full
            # x(0) load (SBUF deps are tile-granular)
            x0h = []

            def load_x0h(t):
                lo = max(0, TR * t - 1) * WP
                hi = (TR * t + TR + 1) * WP + 2
                ht = const.tile([128, (TR + 2) * WP + 2], f32r, tag=f"x0h{t}")
                nc.sync.dma_start(ht[:, 0:hi - lo], x_d[0, :, lo:hi])
                x0h.append((ht, lo))

            load_x0h(0)
            pall = const.tile([128, 280], f32, tag="pall")
            nc.sync.dma_start(pall[:], pall_d[:])
            load_x0h(1)
            load_x0h(2)
            p1 = pall
            pw_t = [pall[:, 24:152], pall[:, 152:280]]
            p2_t = [pall[:, 14:19], pall[:, 19:24]]

            def load_x(b, skip=0):
                # skip>0: sample-0's first rows live in the head tiles, so
                # its full-tile load can omit them (less startup DMA)
                x_sb = xpool.tile([128, HWP], f32r, tag="x")
                nc.sync.dma_start(x_sb[:, skip:], x_d[b, :, skip:])
                return x_sb

            xq = {0: load_x(0, skip=23 * WP), 1: load_x(1)}
            xq[2] = load_x(2)
            ident = const.tile([128, 128], f32, tag="ident")
            make_identity(nc, ident[:])

            # depthwise tap diagonals (raw weights; BN1 scale rides on the
            # ACT epilogue's per-partition `scale` operand instead, so the
            # first matmul doesn't wait on the affine chain)
            dmats = []
            for ti, (dh, dw) in enumerate(TAPS):
                tap_col = (dh + 1) * 3 + (dw + 1)
                d = const.tile([128, 128], f32r, tag=f"d{ti}")
                nc.vector.tensor_scalar(
                    d[:], ident[:], p1[:, tap_col:tap_col + 1], None, Alu.mult
                )
                dmats.append(d)

            s1, bias1 = _affine(
                nc, const, "a1", p1[:, 13:14], p1[:, 10:11], p1[:, 11:12],
                p1[:, 12:13], p1[:, 9:10],
            )

            # pointwise weight setup is deferred (see setup_pw below) so the
            # PE's first instructions are sample-0 depthwise matmuls, not
            # transposes stuck behind the DVE affine chain.
            pwT = const.tile([128, 256], f32, tag="pwT")
            T2 = []

            def setup_pw(pstr):
                for ob in range(2):
                    sl = slice(ob * 128, (ob + 1) * 128)
                    pwv = pw_t[ob]
                    p2 = p2_t[ob]
                    s2, t2 = _affine(
                        nc, const, f"a2{ob}", p2[:, 4:5], p2[:, 1:2],
                        p2[:, 2:3], p2[:, 3:4], p2[:, 0:1],
                    )
                    T2.append(t2)
                    pws = const.tile([128, 128], f32, tag=f"pws{ob}")
                    nc.vector.tensor_scalar(pws[:], pwv, s2[:], None, Alu.mult)
                    pt = pstr.tile([128, 2, TR, 64], f32, tag="pspw")
                    ptv = pt.rearrange("p a r w -> p (a r w)")[:, 0:128]
                    nc.tensor.transpose(ptv, pws[:], ident[:])
                    nc.vector.tensor_copy(pwT[:, sl], ptv)

            # scratch target for the fused max-accum ops (value discarded)
            scr = const.tile([128, 2, TR, 64], f32, tag="scr")

            with (
                tc.tile_pool(name="psdw", bufs=2, space="PSUM") as psdw,
                tc.tile_pool(name="pspw", bufs=3, space="PSUM") as pspw,
            ):
                state = {}

                def make_xv(x_sb):
                    # per-dw base views: view[dw][r, 0:56] = x[r, w+dw] with
                    # zero padding supplied by the shared pad column
                    return {
                        dw: x_sb[:, 1 + dw:1 + dw + H * WP]
                        .rearrange("p (h w) -> p h w", w=WP)
                        for dw in (-1, 0, 1)
                    }

                def dw_stage(b):
                    """Generator: one yield per DW pixel-tile group."""
                    x_sb = xq.pop(b)
                    if b + 3 < BPC:
                        xq[b + 3] = load_x(b + 3)
                    xv = make_xv(x_sb)
                    y_sb = ypool.tile([128, HW], f32r, tag="y")
                    y4 = y_sb.rearrange("p (t r w) -> p t r w", t=NT, r=TR)
                    mp = stats.tile([128, 8], f32, tag="mp1")
                    for t in range(NT):
                        ps = psdw.tile([128, TR, 64], f32, tag="psdw")
                        r0 = TR * t
                        for ti, (dh, dw) in enumerate(TAPS):
                            a = max(r0, -dh)
                            bb = min(r0 + TR, 56 - max(0, dh))
                            if b == 0 and t < 3:
                                # sample-0 head start: read from the small
                                # per-group chunk tile instead of the full x
                                ht, lo = x0h[t]
                                base = (a + dh) * WP + 1 + dw - lo
                                rhs = (
                                    ht[:, base:base + (bb - a) * WP]
                                    .rearrange("p (h w) -> p h w", w=WP)[:, :, 0:56]
                                )
                            else:
                                rhs = xv[dw][:, a + dh:bb + dh, 0:56]
                            nc.tensor.matmul(
                                ps[:, a - r0:bb - r0, 0:56],
                                dmats[ti][:],
                                rhs,
                                start=(ti == 0),
                                stop=(ti == len(TAPS) - 1),
                            )
                        nc.scalar.activation(
                            y4[:, t],
                            ps[:, :, 0:56],
                            Act.Relu,
                            bias=bias1[:],
                            scale=s1[:],
                        )
                        nc.vector.tensor_scalar(
                            scr[:, 0, :, 0:56],
                            y4[:, t],
                            0.0, None, Alu.add,
                            op1=Alu.max,
                            accum_out=mp[:, t:t + 1],
                        )
                        yield
                    ymax = stats.tile([128, 1], f32, tag="ymax")
                    nc.vector.tensor_reduce(ymax[:], mp[:, 0:NT], axis=AxL.X, op=Alu.max)
                    mask1 = stats.tile([128, 1], f32, tag="mask1")
                    nc.vector.tensor_scalar(mask1[:], ymax[:], DW_T, None, Alu.is_ge)
                    lm = lmpool.tile([128, 256], f32r, tag="lm")
                    nc.vector.tensor_scalar(
                        lm[:, 0:128], pwT[:, 0:128], mask1[:], None, Alu.mult
                    )
                    nc.vector.tensor_scalar(
                        lm[:, 128:256], pwT[:, 128:256], mask1[:], None, Alu.mult
                    )
                    state[b] = (y4, lm)

                def pw_stage(b):
                    """Generator: one yield per PW psum pair-group."""
                    y4, lm = state.pop(b)
                    for ob in range(2):
                        z_sb = zpool.tile([128, HW], f32, tag="z")
                        z4 = z_sb.rearrange("p (t r w) -> p t r w", t=NT, r=TR)
                        mpz = stats.tile([128, 4], f32, tag="mpz")
                        for k in range(4):
                            n_t = min(2, NT - 2 * k)
                            ps = pspw.tile([128, 2, TR, 64], f32, tag="pspw")
                            for half in range(n_t):
                                t = 2 * k + half
                                nc.tensor.matmul(
                                    ps[:, half, :, 0:56],
                                    lm[:, ob * 128:(ob + 1) * 128],
                                    y4[:, t],
                                    start=True,
                                    stop=True,
                                )
                            nc.scalar.activation(
                                z4[:, 2 * k:2 * k + n_t],
                                ps[:, 0:n_t, :, 0:56],
                                Act.Relu,
                                bias=T2[ob][:],
                            )
                            if b == BPC - 1 and ob == 1 and k == 3:
                                # tail chain: take the max from PSUM so it
                                # runs concurrent with the ACT epilogue
                                # (max(psum)+T2 vs 0.001 decides identically
                                # to max(relu(psum+T2)) since T2 shifts both)
                                nc.vector.tensor_scalar(
                                    scr[:, 0:n_t, :, 0:56],
                                    ps[:, 0:n_t, :, 0:56],
                                    T2[ob][:], None, Alu.add,
                                    op1=Alu.max,
                                    accum_out=mpz[:, k:k + 1],
                                )
                            else:
                                nc.vector.tensor_scalar(
                                    scr[:, 0:n_t, :, 0:56],
                                    z4[:, 2 * k:2 * k + n_t],
                                    0.0, None, Alu.add,
                                    op1=Alu.max,
                                    accum_out=mpz[:, k:k + 1],
                                )
                            yield
                        zmax = stats.tile([128, 1], f32, tag="zmax")
                        nc.vector.tensor_reduce(zmax[:], mpz[:], axis=AxL.X, op=Alu.max)
                        maskz = stats.tile([128, 1], f32, tag="maskz")
                        nc.vector.tensor_scalar(maskz[:], zmax[:], PW_T, None, Alu.is_ge)
                        # mask-multiply split across DVE and the idle GpSimd so
                        # the per-sample store chain is two parallel pipelines
                        for c in range(4):
                            seg = z_sb[:, c * 784:(c + 1) * 784]
                            last = b == BPC - 1 and ob == 1
                            eng = nc.vector if (c % 2 == 0 or last) else nc.gpsimd
                            eng.tensor_scalar(seg, seg, maskz[:], None, Alu.mult)
                            nc.sync.dma_start(
                                z_d[b, ob * 128:(ob + 1) * 128,
                                    c * 784:(c + 1) * 784],
                                seg,
                            )

                # software pipeline with group-level interleave: DW(b+1)
                # groups are traced between PW(b) groups so the PE always has
                # dense work and the PW mask latency is fully hidden.
                def drain(g, n=1000):
                    for _ in range(n):
                        try:
                            next(g)
                        except StopIteration:
                            return True
                    return False

                g0 = dw_stage(0)
                drain(g0, 5)
                setup_pw(pspw)
                drain(g0)
                for b in range(BPC):
                    gdw = dw_stage(b + 1) if b + 1 < BPC else None
                    gpw = pw_stage(b)
                    while True:
                        done_dw = gdw is None or drain(gdw, 1)
                        done_pw = drain(gpw, 2)
                        if done_pw and done_dw:
                            break

    nc.compile()
    return nc


_NC_CACHE = None


def make_in_maps(inputs):
    def f(name):
        return np.asarray(inputs[name], dtype=np.float32)

    x = f("x").reshape(B, CIN, H, W)
    xp = np.zeros((B, CIN, HWP), dtype=np.float32)
    xp[:, :, :H * WP].reshape(B, CIN, H, WP)[:, :, :, 1:] = x
    p2 = np.concatenate(
        [f(k).reshape(COUT, 1)
         for k in ("pw_b", "bn2_gamma", "bn2_beta", "bn2_mean", "bn2_var")],
        axis=1,
    )
    pww = f("pw_w").reshape(COUT, CIN)
    pall = np.concatenate(
        [f("dw_w").reshape(CIN, 9)]
        + [f(k).reshape(CIN, 1)
           for k in ("dw_b", "bn1_gamma", "bn1_beta", "bn1_mean", "bn1_var")]
        + [p2[0:128], p2[128:256], pww[0:128], pww[128:256]],
        axis=1,
    )
    base = {"pall": np.ascontiguousarray(pall)}
    return [
        {"x": np.ascontiguousarray(xp[i * BPC:(i + 1) * BPC]), **base}
        for i in range(N_CORES)
    ]


def kernel(**inputs) -> np.ndarray:
    global _NC_CACHE
    if _NC_CACHE is None:
        _NC_CACHE = build()
    nc = _NC_CACHE
    in_maps = make_in_maps(inputs)
    res = run_bass_kernel_spmd(nc, in_maps, core_ids=list(range(N_CORES)))
    out = np.concatenate([r["z"] for r in res.results], axis=0)
    return out.reshape(B, COUT, H, W)


if __name__ == "__main__":
    build()
    print("build ok")
